# revision 1
# baseline (speedup 1.0000x reference)
"""AttnVLAD Trainium2 kernel.

Shapes (hardcoded): x [16, 512, 8192] f32, centers [1, 512, 64] f32,
alpha [1] f32, cluster_weights [1, 1, 64] f32 -> out [16, 32768] f32.

Sharding: data-parallel over batch B=16 across 8 cores (2 batches/core);
centers/alpha/cluster_weights replicated.

Per-core dataflow (per batch, streaming n in chunks of 512):
  - x chunk loaded as fp16 via SWDGE cast-DMA (HBM f32 -> SBUF fp16).
  - mm1: score^T[n,k] = x_chunk^T @ q_alpha  (fp16, PSUM f32), where
    q_alpha = alpha * l2norm(centers) computed on-device once.
  - softmax over k read straight from PSUM: DVE reduce_max(negate) ->
    ACT exp(bias=-max, accum_out=colsum) -> DVE reciprocal ->
    DVE per-partition-scalar mul -> prob fp16 (renormalized per column).
  - x^T via PE transposes (fp16) -> PSUM -> evac to SBUF (DVE/ACT split).
  - mm2: desc_raw^T[k,d] += prob^T @ x^T accumulated over all n in PSUM;
    denominators accumulated the same way against a ones vector.
  - finalize: desc = desc_raw/denom - centers, intra-L2-norm over d,
    cluster_weights scale, global L2 norm, transpose back to [d,k], DMA out.
"""

import os
import numpy as np

import concourse.bass as bass
import concourse.tile as tile
from concourse import bacc, mybir
from concourse.bass_utils import run_bass_kernel_spmd

F32 = mybir.dt.float32
F16 = mybir.dt.float16
AF = mybir.ActivationFunctionType

B, D, K, N = 16, 512, 64, 8192
NCORES = 8
B_LOC = B // NCORES          # 2 batches per core
CHUNK = 512                  # n columns per chunk
NCH = N // CHUNK             # 16 chunks
DJ = D // 128                # 4 d-chunks
NI = CHUNK // 128            # 4 n-tiles per chunk
EPS = 1e-6

_run_results = [None]        # stash for test harness introspection


def _build(ablate=frozenset(), group=4, reps=1):
    """group: number of CHUNKs covered by one x DMA (bigger -> longer
    per-partition DRAM runs: run bytes = CHUNK*group/NCH... = CHUNK*4B*...)."""
    nc = bacc.Bacc("TRN2", target_bir_lowering=False, debug=False)

    x_in = nc.dram_tensor("x_sh", [B_LOC, D, N], F32, kind="ExternalInput").ap()
    cen_in = nc.dram_tensor("centers", [D, K], F32, kind="ExternalInput").ap()
    alpha_in = nc.dram_tensor("alpha", [1, 1], F32, kind="ExternalInput").ap()
    cw_in = nc.dram_tensor("cw", [K, 1], F32, kind="ExternalInput").ap()
    id16_in = nc.dram_tensor("ident16", [128, 128], F16, kind="ExternalInput").ap()
    id32_in = nc.dram_tensor("ident32", [128, 128], F32, kind="ExternalInput").ap()
    out = nc.dram_tensor("out", [B_LOC, D * K], F32, kind="ExternalOutput").ap()

    GN = CHUNK * group          # n-columns per x DMA
    NG = N // GN                # DMAs per batch
    dma_f32 = ("dmaf32g" in ablate) or ("dmaf32s" in ablate)
    no_dma = "dma" in ablate

    with tile.TileContext(nc) as tc:
        with (
            tc.tile_pool(name="const", bufs=1) as cpool,
            tc.tile_pool(name="x", bufs=max(2, 8 // group)) as xpool,
            tc.tile_pool(name="xtsb", bufs=4) as xtsbpool,
            tc.tile_pool(name="prob", bufs=3) as ppool,
            tc.tile_pool(name="stats", bufs=3) as stpool,
            tc.tile_pool(name="fin", bufs=2) as fpool,
            tc.tile_pool(name="sc_ps", bufs=3, space="PSUM") as scps,
            tc.tile_pool(name="xt_ps", bufs=3,
                         space="PSUM") as xtps,
            tc.tile_pool(name="acc_ps", bufs=1, space="PSUM") as accps,
        ):
            # ---------------- constants / setup ----------------
            id16 = cpool.tile([128, 128], F16)
            nc.gpsimd.dma_start(id16[:], id16_in)
            id32 = cpool.tile([128, 128], F32)
            nc.gpsimd.dma_start(id32[:], id32_in)
            ct = cpool.tile([128, DJ * K], F32)       # centers, free=(j,k)
            nc.gpsimd.dma_start(
                ct[:].rearrange("p (j k) -> p j k", k=K),
                cen_in.rearrange("(j p) k -> p j k", p=128),
            )
            alpha_sb = cpool.tile([1, 1], F32)
            nc.gpsimd.dma_start(alpha_sb[:], alpha_in)
            cw_sb = cpool.tile([K, 1], F32)
            nc.gpsimd.dma_start(cw_sb[:], cw_in)
            ones16 = cpool.tile([128, 1], F16)
            nc.vector.memset(ones16[:], 1.0)
            onesK = cpool.tile([K, 1], F32)
            nc.vector.memset(onesK[:], 1.0)
            ones_row = cpool.tile([1, K], F32)
            nc.vector.memset(ones_row[:], 1.0)

            # centers^T [k, d]
            ctT_ps = scps.tile([K, D], F32, tag="sc")
            for j in range(DJ):
                nc.tensor.transpose(
                    ctT_ps[:, j * 128:(j + 1) * 128],
                    ct[:, j * K:(j + 1) * K],
                    id32[:],
                )
            cenT = cpool.tile([K, D], F32)
            nc.vector.tensor_copy(cenT[:], ctT_ps[:])

            # column norms of centers -> q scale = alpha / max(norm, 1e-12)
            csq = cpool.tile([K, D], F32)
            cssq = cpool.tile([K, 1], F32)
            nc.scalar.activation(csq[:], cenT[:], AF.Square, accum_out=cssq[:])
            cnorm = cpool.tile([K, 1], F32)
            nc.scalar.activation(cnorm[:], cssq[:], AF.Sqrt)
            nc.vector.tensor_scalar_max(cnorm[:], cnorm[:], 1e-12)
            crcp = cpool.tile([K, 1], F32)
            nc.vector.reciprocal(crcp[:], cnorm[:])
            ab_ps = scps.tile([K, 1], F32, tag="sc")
            nc.tensor.matmul(ab_ps[:], ones_row[:], alpha_sb[:], start=True, stop=True)
            ab = cpool.tile([K, 1], F32)
            nc.vector.tensor_copy(ab[:], ab_ps[:])
            qscale = cpool.tile([K, 1], F32)
            nc.vector.tensor_mul(qscale[:], crcp[:], ab[:])
            qaT = cpool.tile([K, D], F32)
            nc.vector.tensor_scalar_mul(qaT[:], cenT[:], qscale[:])
            qa_ps = scps.tile([128, DJ * K], F32, tag="sc")
            for j in range(DJ):
                nc.tensor.transpose(
                    qa_ps[:, j * K:(j + 1) * K],
                    qaT[:, j * 128:(j + 1) * 128],
                    id32[0:K, 0:K],
                )
            qa = cpool.tile([128, DJ * K], F16)
            nc.vector.tensor_copy(qa[:], qa_ps[:])

            # ---------------- ablation shared tiles ----------------
            probn_s = x_shared = sc_shared = xt_shared = None
            rcp_s = rcp16_s = None
            if "softmax" in ablate:
                probn_s = cpool.tile([128, NI * K], F16, name="probn_s")
                nc.vector.memset(probn_s[:], 0.01)
                rcp_s = cpool.tile([128, NI], F32, name="rcp_s")
                nc.vector.memset(rcp_s[:], 1.0)
                rcp16_s = cpool.tile([128, NI], F16, name="rcp16_s")
                nc.vector.memset(rcp16_s[:], 1.0)
            if no_dma or dma_f32:
                x_shared = cpool.tile([128, DJ * CHUNK], F16, name="x_shared")
                nc.vector.memset(x_shared[:], 0.25)
            if "mm1" in ablate:
                sc_shared = accps.tile([128, NI * K], F32, name="sc_shared")
                nc.vector.memset(sc_shared[:], 1.0)
            if "trans" in ablate:
                xt_shared = cpool.tile([128, NI * D], F16, name="xt_shared")
                nc.vector.memset(xt_shared[:], 0.125)

            # ---------------- main loop (optionally repeated for timing) ----
            import contextlib
            loop_cm = tc.For_i(0, reps, 1) if reps > 1 else contextlib.nullcontext()
            with loop_cm:
              for b in range(B_LOC):
                desc_ps = accps.tile([K, D], F32, tag="desc", name="desc_ps")
                den_ps = accps.tile([2 * K, 1], F32, tag="den", name="den_ps")
                if "mm2" in ablate:
                    nc.vector.memset(desc_ps[:], 0.5)
                    nc.vector.memset(den_ps[:], 1.0)
                for cc in range(NG):
                    if no_dma:
                        x_g = None
                    else:
                        xdt = F32 if dma_f32 else F16
                        x_g = xpool.tile([128, DJ * GN], xdt, tag="x", name="x_g")
                        eng = nc.sync if "dmaf32s" in ablate else nc.gpsimd
                        eng.dma_start(
                            x_g[:].rearrange("p (j n) -> p j n", n=GN),
                            x_in[b, :, cc * GN:(cc + 1) * GN].rearrange(
                                "(j p) n -> p j n", p=128),
                        )
                    for c2 in range(group):
                        c = cc * group + c2

                        def xblk(j, i):
                            if no_dma or dma_f32:
                                return x_shared[:, j * CHUNK + i * 128:
                                                j * CHUNK + (i + 1) * 128]
                            return x_g[:, j * GN + c2 * CHUNK + i * 128:
                                       j * GN + c2 * CHUNK + (i + 1) * 128]

                        # mm1: score^T [n, k] per n-tile i
                        if "mm1" not in ablate:
                            score_ps = scps.tile([128, NI * K], F32, tag="sc",
                                                 name="score_ps")
                            for i in range(NI):
                                for j in range(DJ):
                                    nc.tensor.matmul(
                                        score_ps[:, i * K:(i + 1) * K],
                                        xblk(j, i),
                                        qa[:, j * K:(j + 1) * K],
                                        start=(j == 0),
                                        stop=(j == DJ - 1),
                                    )
                        else:
                            score_ps = sc_shared
                        # x^T transposes into two 2-bank tiles; one big evac each
                        if "trans" not in ablate:
                            xt_sb = xtsbpool.tile([128, NI * D], F16, tag="xtsb",
                                                  name="xt_sb")
                            for i in range(NI):
                                xt_ps = xtps.tile([128, D], F16, tag="xt",
                                                  name="xt_ps")
                                for j in range(DJ):
                                    nc.tensor.transpose(
                                        xt_ps[:, j * 128:(j + 1) * 128],
                                        xblk(j, i),
                                        id16[:],
                                    )
                                nc.vector.tensor_copy(
                                    xt_sb[:, i * D:(i + 1) * D], xt_ps[:])
                        else:
                            xt_sb = xt_shared
                        # softmax over k (free dim), straight off PSUM
                        if "softmax" not in ablate:
                            nbias = stpool.tile([128, NI], F32, tag="nbias",
                                                name="nbias")
                            nc.vector.tensor_reduce(
                                nbias[:],
                                score_ps[:].rearrange("p (i k) -> p i k", k=K),
                                axis=mybir.AxisListType.X,
                                op=mybir.AluOpType.max,
                                negate=True,
                            )
                            prob = ppool.tile([128, NI * K], F16, tag="prob",
                                              name="prob")
                            colsum = stpool.tile([128, NI], F32, tag="colsum",
                                                 name="colsum")
                            for i in range(NI):
                                nc.scalar.activation(
                                    prob[:, i * K:(i + 1) * K],
                                    score_ps[:, i * K:(i + 1) * K],
                                    AF.Exp,
                                    bias=nbias[:, i:i + 1],
                                    accum_out=colsum[:, i:i + 1],
                                )
                            rcp = stpool.tile([128, NI], F32, tag="rcp",
                                              name="rcp")
                            nc.vector.reciprocal(rcp[:], colsum[:])
                            probn = ppool.tile([128, NI * K], F16, tag="probn",
                                               name="probn")
                            for i in range(NI):
                                nc.vector.tensor_scalar_mul(
                                    probn[:, i * K:(i + 1) * K],
                                    prob[:, i * K:(i + 1) * K],
                                    rcp[:, i:i + 1],
                                )
                        else:
                            probn = probn_s
                        # mm2 + denominator accumulation
                        if "mm2" not in ablate:
                            for i in range(NI):
                                first = (c == 0 and i == 0)
                                last = (c == NCH - 1 and i == NI - 1)
                                nc.tensor.matmul(
                                    desc_ps[:],
                                    probn[:, i * K:(i + 1) * K],
                                    xt_sb[:, i * D:(i + 1) * D],
                                    start=first,
                                    stop=last,
                                    skip_group_check=True,
                                )
                                if i % 2 == 0:
                                    nc.tensor.matmul(
                                        den_ps[:],
                                        probn[:, i * K:(i + 2) * K],
                                        ones16[:],
                                        start=first,
                                        stop=(c == NCH - 1 and i == NI - 2),
                                        skip_group_check=True,
                                    )

                # ---------------- finalize batch ----------------
                descT = fpool.tile([K, D], F32, tag="descT")
                nc.vector.tensor_copy(descT[:], desc_ps[:])
                den2 = fpool.tile([2 * K, 1], F32, tag="den2_sb")
                nc.vector.tensor_copy(den2[:], den_ps[:])
                denc = fpool.tile([K, 2], F32, tag="denc_sb")
                nc.vector.tensor_copy(denc[:, 0:1], den2[0:K, :])
                nc.gpsimd.dma_start(denc[:, 1:2], den2[K:2 * K, :])
                den = fpool.tile([K, 1], F32, tag="den_sb")
                nc.vector.tensor_reduce(den[:], denc[:],
                                        axis=mybir.AxisListType.X,
                                        op=mybir.AluOpType.add)
                nc.vector.tensor_scalar_max(den[:], den[:], EPS)
                rden = fpool.tile([K, 1], F32, tag="rden")
                nc.vector.reciprocal(rden[:], den[:])
                desc2 = fpool.tile([K, D], F32, tag="desc2")
                nc.vector.tensor_scalar_mul(desc2[:], descT[:], rden[:])
                nc.vector.tensor_sub(desc2[:], desc2[:], cenT[:])
                sq = fpool.tile([K, D], F32, tag="sq")
                ssq = fpool.tile([K, 1], F32, tag="ssq")
                nc.scalar.activation(sq[:], desc2[:], AF.Square, accum_out=ssq[:])
                snorm = fpool.tile([K, 1], F32, tag="snorm")
                nc.scalar.activation(snorm[:], ssq[:], AF.Sqrt)
                nc.vector.tensor_scalar_max(snorm[:], snorm[:], 1e-12)
                rn = fpool.tile([K, 1], F32, tag="rn")
                nc.vector.reciprocal(rn[:], snorm[:])
                scl = fpool.tile([K, 1], F32, tag="scl")
                nc.vector.tensor_mul(scl[:], rn[:], cw_sb[:])
                descn = fpool.tile([K, D], F32, tag="descn")
                nc.vector.tensor_scalar_mul(descn[:], desc2[:], scl[:])
                sq2 = fpool.tile([K, D], F32, tag="sq2")
                gss = fpool.tile([K, 1], F32, tag="gss")
                nc.scalar.activation(sq2[:], descn[:], AF.Square, accum_out=gss[:])
                g_ps = scps.tile([1, 1], F32, tag="sc")
                nc.tensor.matmul(g_ps[:], gss[:], onesK[:], start=True, stop=True)
                gval = fpool.tile([1, 1], F32, tag="gval")
                nc.vector.tensor_copy(gval[:], g_ps[:])
                nc.scalar.activation(gval[:], gval[:], AF.Sqrt)
                nc.vector.tensor_scalar_max(gval[:], gval[:], 1e-12)
                grc = fpool.tile([1, 1], F32, tag="grc")
                nc.vector.reciprocal(grc[:], gval[:])
                gb_ps = scps.tile([K, 1], F32, tag="sc")
                nc.tensor.matmul(gb_ps[:], ones_row[:], grc[:], start=True, stop=True)
                gb = fpool.tile([K, 1], F32, tag="gb")
                nc.vector.tensor_copy(gb[:], gb_ps[:])
                descf = fpool.tile([K, D], F32, tag="descf")
                nc.vector.tensor_scalar_mul(descf[:], descn[:], gb[:])
                o_ps = scps.tile([128, DJ * K], F32, tag="sc")
                for j in range(DJ):
                    nc.tensor.transpose(
                        o_ps[:, j * K:(j + 1) * K],
                        descf[:, j * 128:(j + 1) * 128],
                        id32[0:K, 0:K],
                    )
                out_sb = fpool.tile([128, DJ * K], F32, tag="out_sb")
                nc.vector.tensor_copy(out_sb[:], o_ps[:])
                nc.gpsimd.dma_start(
                    out[b].rearrange("(j p k) -> p j k", p=128, k=K),
                    out_sb[:].rearrange("p (j k) -> p j k", k=K),
                )

    nc.compile()
    return nc


_NC_CACHE = [None]


def kernel(x, centers, alpha, cluster_weights):
    if _NC_CACHE[0] is None:
        _NC_CACHE[0] = _build()
    nc = _NC_CACHE[0]

    x = np.ascontiguousarray(np.asarray(x, dtype=np.float32))
    cen = np.ascontiguousarray(np.asarray(centers, dtype=np.float32).reshape(D, K))
    al = np.asarray(alpha, dtype=np.float32).reshape(1, 1)
    cw = np.ascontiguousarray(np.asarray(cluster_weights, dtype=np.float32).reshape(K, 1))
    id16 = np.eye(128, dtype=np.float16)
    id32 = np.eye(128, dtype=np.float32)

    in_maps = []
    for core in range(NCORES):
        in_maps.append({
            "x_sh": x[core * B_LOC:(core + 1) * B_LOC],
            "centers": cen,
            "alpha": al,
            "cw": cw,
            "ident16": id16,
            "ident32": id32,
        })

    res = run_bass_kernel_spmd(
        nc, in_maps, core_ids=list(range(NCORES)), trace=False
    )
    _run_results[0] = res
    out = np.concatenate([r["out"] for r in res.results], axis=0)
    return out.astype(np.float32)


def _make_in_maps(x, centers, alpha, cluster_weights):
    x = np.ascontiguousarray(np.asarray(x, dtype=np.float32))
    cen = np.ascontiguousarray(np.asarray(centers, dtype=np.float32).reshape(D, K))
    al = np.asarray(alpha, dtype=np.float32).reshape(1, 1)
    cw = np.ascontiguousarray(np.asarray(cluster_weights, dtype=np.float32).reshape(K, 1))
    id16 = np.eye(128, dtype=np.float16)
    id32 = np.eye(128, dtype=np.float32)
    return [
        {
            "x_sh": x[core * B_LOC:(core + 1) * B_LOC],
            "centers": cen,
            "alpha": al,
            "cw": cw,
            "ident16": id16,
            "ident32": id32,
        }
        for core in range(NCORES)
    ]


def timed_run(x, centers, alpha, cluster_weights, iters=6):
    """Mirror of bass2jax.run_bass_via_pjrt that jits once, stages inputs on
    device, and re-executes to measure steady-state per-iteration wall time.
    Returns (full_output, list_of_iter_seconds)."""
    import time
    import jax
    from jax.sharding import Mesh, PartitionSpec, NamedSharding
    from jax.experimental.shard_map import shard_map
    from concourse import bass2jax, mybir as mb

    if _NC_CACHE[0] is None:
        _NC_CACHE[0] = _build()
    nc = _NC_CACHE[0]
    bass2jax.install_neuronx_cc_hook()

    in_maps = _make_in_maps(x, centers, alpha, cluster_weights)

    partition_name = nc.partition_id_tensor.name if nc.partition_id_tensor else None
    in_names, out_names, out_avals, zero_outs = [], [], [], []
    for alloc in nc.m.functions[0].allocations:
        if not isinstance(alloc, mb.MemoryLocationSet):
            continue
        name = alloc.memorylocations[0].name
        if alloc.kind == "ExternalInput":
            if name != partition_name:
                in_names.append(name)
        elif alloc.kind == "ExternalOutput":
            out_names.append(name)
            shape = tuple(alloc.tensor_shape)
            dtype = mb.dt.np(alloc.dtype)
            out_avals.append(jax.core.ShapedArray(shape, dtype))
            zero_outs.append(np.zeros(shape, dtype))
    n_params = len(in_names)
    n_outs = len(out_avals)
    all_in_names = list(in_names) + list(out_names)
    if partition_name is not None:
        all_in_names.append(partition_name)

    def _one(ins, outs):
        operands = list(ins) + list(outs)
        if partition_name is not None:
            operands.append(bass2jax.partition_id_tensor())
        return tuple(bass2jax._bass_exec_p.bind(
            *operands,
            out_avals=tuple(out_avals),
            in_names=tuple(all_in_names),
            out_names=tuple(out_names),
            lowering_input_output_aliases=(),
            sim_require_finite=True,
            sim_require_nnan=True,
            nc=nc,
        ))

    def make_body(rep):
        def _body(*args):
            ins = args[:n_params]
            outs = args[n_params:]
            for _ in range(rep):
                outs = _one(ins, outs)
            return outs
        return _body

    devices = jax.devices()[:NCORES]
    mesh = Mesh(np.asarray(devices), ("core",))
    spec = PartitionSpec("core")
    in_specs = (spec,) * (n_params + n_outs)
    out_specs = (spec,) * n_outs

    per_core = [[np.asarray(m[name]) for name in in_names] for m in in_maps]
    concat_in = [
        np.concatenate([per_core[c][i] for c in range(NCORES)], axis=0)
        for i in range(n_params)
    ]
    concat_zeros = [
        np.zeros((NCORES * z.shape[0], *z.shape[1:]), z.dtype) for z in zero_outs
    ]
    sharding = NamedSharding(mesh, spec)
    staged = [jax.device_put(a, sharding) for a in concat_in]
    staged_zeros = [jax.device_put(a, sharding) for a in concat_zeros]
    jax.block_until_ready(staged)

    sharded = jax.jit(
        shard_map(make_body(1), mesh=mesh, in_specs=in_specs,
                  out_specs=out_specs, check_rep=False), keep_unused=True)

    # warm-up (compiles) + correctness output
    out_arrs = sharded(*staged, *staged_zeros)
    jax.block_until_ready(out_arrs)
    oi = out_names.index("out")
    full_out = np.asarray(out_arrs[oi]).reshape(B, D * K).astype(np.float32)

    # chained async dispatches: outputs of exec i feed exec i+1, block once.
    def run_chain(rep):
        t0 = time.perf_counter()
        outs = tuple(staged_zeros)
        for _ in range(rep):
            outs = sharded(*staged, *outs)
        jax.block_until_ready(outs)
        return time.perf_counter() - t0

    rep_lo, rep_hi = 1, 41
    tlo, thi = [], []
    for _ in range(iters):
        tlo.append(run_chain(rep_lo))
        thi.append(run_chain(rep_hi))
    per_exec = (min(thi) - min(tlo)) / (rep_hi - rep_lo)
    return full_out, {"per_exec_s": per_exec, "lo": tlo, "hi": thi}



# revision 2
# speedup vs baseline: 2.7766x; 2.7766x over previous
"""AttnVLAD Trainium2 kernel (optimized).

Shapes (hardcoded): x [16, 512, 8192] f32, centers [1, 512, 64] f32,
alpha [1] f32, cluster_weights [1, 1, 64] f32 -> out [16, 32768] f32.

Sharding: data-parallel over batch B=16 across 8 cores (2 batches/core);
centers/alpha/cluster_weights replicated.

v2 structural changes vs v1 (all cost-model driven):
  - exp WITHOUT accum_out (saves 187ns ACT read-accum per op); colsum
    computed on DVE from the fp16 prob in SBUF (one segmented reduce per
    chunk).
  - softmax column renorm folded into the x^T evacuation:
    desc = sum_n e_kn * (r_n x_dn) with r = 1/colsum, so prob is used
    unnormalized and the per-column 1/colsum scales x^T during PSUM->SBUF
    evacuation (which we had to do anyway).  Removes 4 DVE muls/chunk.
  - x^T evacuation split between DVE and ACT to balance engine load.
  - mm2 reoriented: out desc[d,k] (128 partitions) with lhsT=x^T tile,
    rhs=prob -> streams 64 instead of 512 cols per instruction.
    den row [1,64] accumulated via lhsT=r16.
  - finalize entirely in [d,k] layout: per-k factors computed as [1,K]
    rows, broadcast across partitions via outer-product matmuls
    (lhsT=ones[1,128]); no PE transposes, no ACT table swaps (only Exp,
    Copy, Square, Ln used on ACT; rsqrt via exp(-ln/2), one act table).
"""

import numpy as np

import concourse.bass as bass
import concourse.tile as tile
from concourse import bacc, mybir
from concourse.bass_utils import run_bass_kernel_spmd

F32 = mybir.dt.float32
F16 = mybir.dt.float16
AF = mybir.ActivationFunctionType
ALU = mybir.AluOpType

B, D, K, N = 16, 512, 64, 8192
NCORES = 8
B_LOC = B // NCORES          # 2 batches per core
CHUNK = 512                  # n columns per chunk
NCH = N // CHUNK             # 16 chunks
DJ = D // 128                # 4 d-chunks
NI = CHUNK // 128            # 4 n-tiles per chunk
EPS = 1e-6

_run_results = [None]


def _build(reps=1, group=8, evac_dve=2, debug_acc=False, stagger=1,
           ablate="", xdma="g16", kd=1, xbar=0, layout="i"):
    """evac_dve: base count of the NI=4 x^T evacuations on DVE (rest ACT).
    debug_acc: add a dbg output with raw desc accumulator + den per batch."""
    nc = bacc.Bacc("TRN2", target_bir_lowering=False, debug=False)

    x_in = nc.dram_tensor("x_sh", [B_LOC, D, N], F32, kind="ExternalInput").ap()
    cen_in = nc.dram_tensor("centers", [D, K], F32, kind="ExternalInput").ap()
    alpha_in = nc.dram_tensor("alpha", [1, 1], F32, kind="ExternalInput").ap()
    cw_in = nc.dram_tensor("cw", [1, K], F32, kind="ExternalInput").ap()
    id16_in = nc.dram_tensor("ident16", [128, 128], F16, kind="ExternalInput").ap()
    id32_in = nc.dram_tensor("ident32", [128, 128], F32, kind="ExternalInput").ap()
    out = nc.dram_tensor("out", [B_LOC, D * K], F32, kind="ExternalOutput").ap()
    dbg = (nc.dram_tensor("dbg", [B_LOC, 128, (DJ + 1) * K], F32,
                          kind="ExternalOutput").ap() if debug_acc else None)

    GN = CHUNK * group          # n-columns per x DMA
    NG = N // GN                # DMAs per batch

    with tile.TileContext(nc) as tc:
        with (
            tc.tile_pool(name="const", bufs=1) as cpool,
            tc.tile_pool(name="x", bufs=max(2, 8 // group)) as xpool,
            tc.tile_pool(name="xtsb", bufs=3) as xtsbpool,
            tc.tile_pool(name="prob", bufs=4) as ppool,
            tc.tile_pool(name="stats", bufs=3) as stpool,
            tc.tile_pool(name="fin", bufs=2) as fpool,
            tc.tile_pool(name="sc_ps", bufs=2, space="PSUM") as scps,
            tc.tile_pool(name="xt_ps", bufs=2, space="PSUM") as xtps,
            tc.tile_pool(name="fin_ps", bufs=1, space="PSUM") as finps,
            tc.tile_pool(name="acc_ps", bufs=1, space="PSUM") as accps,
        ):
            # ---------------- constants / setup ----------------
            id16 = cpool.tile([128, 128], F16)
            nc.gpsimd.dma_start(id16[:], id16_in)
            id32 = cpool.tile([128, 128], F32)
            nc.gpsimd.dma_start(id32[:], id32_in)
            ct = cpool.tile([128, DJ * K], F32)       # centers, free=(j,k)
            nc.gpsimd.dma_start(
                ct[:].rearrange("p (j k) -> p j k", k=K),
                cen_in.rearrange("(j p) k -> p j k", p=128),
            )
            alpha_sb = cpool.tile([1, 1], F32)
            nc.gpsimd.dma_start(alpha_sb[:], alpha_in)
            cw_sb = cpool.tile([1, K], F32)
            nc.gpsimd.dma_start(cw_sb[:], cw_in)
            ones_c = cpool.tile([128, 1], F32)        # column of ones (lhsT)
            nc.vector.memset(ones_c[:], 1.0)
            ones_r1 = cpool.tile([1, 128], F32)       # row of ones (lhsT bcast)
            nc.vector.memset(ones_r1[:], 1.0)
            ones16 = cpool.tile([128, 1], F16)        # den stationary
            nc.vector.memset(ones16[:], 1.0)

            # Pin the act table to natural_log_exp_and_others (set 6): it
            # contains Exp, Ln, Copy, Square — everything this kernel uses —
            # so bacc's greedy per-func chooser never inserts another load.
            nc.scalar.add_instruction(mybir.InstLoadActFuncSet(
                name=nc.get_next_instruction_name(), ins=[], outs=[],
                act_func_set_id=6))

            # q = alpha * l2norm(centers, axis=d): per-k scale row
            csq = cpool.tile([128, DJ * K], F32)
            nc.scalar.activation(csq[:], ct[:], AF.Square)
            cssq_t = finps.tile([128, 2 * K], F32, tag="fin")
            cssq_ps = cssq_t[0:1, 0:K]
            for j in range(DJ):
                nc.tensor.matmul(cssq_ps, ones_c[:], csq[:, j * K:(j + 1) * K],
                                 start=(j == 0), stop=(j == DJ - 1))
            # qs = alpha / max(sqrt(cssq), 1e-12) via exp(-ln/2)
            # (Ln/Exp share one act table; Sqrt would force table swaps)
            qs = cpool.tile([1, K], F32)
            nc.vector.tensor_scalar_max(qs[:], cssq_ps, 1e-24)
            nc.scalar.activation(qs[:], qs[:], AF.Ln)
            nc.scalar.activation(qs[:], qs[:], AF.Exp, scale=-0.5)
            nc.vector.tensor_scalar_mul(qs[:], qs[:], alpha_sb[0:1, 0:1])
            # broadcast to [128, K] and scale centers -> qa fp16
            qbc_t = finps.tile([128, 2 * K], F32, tag="fin")
            qbc_ps = qbc_t[:, 0:K]
            nc.tensor.matmul(qbc_ps, ones_r1[:], qs[:], start=True, stop=True)
            qa = cpool.tile([128, DJ * K], F16)
            for j in range(DJ):
                nc.vector.tensor_tensor(
                    qa[:, j * K:(j + 1) * K], ct[:, j * K:(j + 1) * K],
                    qbc_ps, op=ALU.mult)

            # ---------------- main loop (software-pipelined) ----------------
            # stage_p(b,c): DMA (per group), mm1, x^T transposes, softmax,
            #   scaled evacuation -> returns chunk state.
            # stage_q(b,c): mm2 + den accumulation (consumes state).
            # Issue order staggers q one chunk behind p so the PE queue
            # always has independent mm1/transpose work ahead of the
            # dependency-waiting mm2.
            xg_state = {}       # (b, cc) -> x_g tile
            desc_state = {}     # b -> (desc_t,)
            x_shared = None
            if ablate == "dma":
                x_shared = cpool.tile([128, DJ * GN], F16, name="x_shared")
                nc.vector.memset(x_shared[:], 0.25)

            def stage_p1(b, c):
                """DMA (per group), mm1, x^T transposes, max, exp."""
                cc, c2 = divmod(c, group)
                if ablate == "dma":
                    xg_state[(b, cc)] = x_shared
                elif c2 == 0:
                    xdt = F16 if xdma.endswith("16") else F32
                    x_g = xpool.tile([128, DJ * GN], xdt, tag="x", name="x_g")
                    eng = nc.gpsimd if xdma.startswith("g") else nc.sync
                    eng.dma_start(
                        x_g[:].rearrange("p (j n) -> p j n", n=GN),
                        x_in[b, :, cc * GN:(cc + 1) * GN].rearrange(
                            "(j p) n -> p j n", p=128),
                    )
                    xg_state[(b, cc)] = x_g
                x_g = xg_state[(b, cc)]
                if ablate == "compute":
                    return None

                def xblk(j, i):
                    return x_g[:, j * GN + c2 * CHUNK + i * 128:
                               j * GN + c2 * CHUNK + (i + 1) * 128]

                # mm1: score [n, k] per n-tile i
                score_ps = scps.tile([128, NI * K], F32, tag="sc",
                                     name="score_ps")
                for i in range(NI):
                    for j in range(DJ):
                        nc.tensor.matmul(
                            score_ps[:, i * K:(i + 1) * K],
                            xblk(j, i),
                            qa[:, j * K:(j + 1) * K],
                            start=(j == 0),
                            stop=(j == DJ - 1),
                        )
                # x^T transposes, xt laid out j-major:
                #   xt[:, j*(NI*128) + i*128 + dd] = x[d=j*128+dd, n-tile i]
                # j < xbar goes through the DMA xbar transpose straight into
                # SBUF (no PE work, no evacuation); the rest via PE + evac.
                xt_sb = xtsbpool.tile([128, NI * D], F16, tag="xtsb",
                                      name="xt_sb")
                JW = NI * 128           # elements per j-region (j-major)

                def xt_off(j, i):
                    if layout == "j":
                        return j * JW + i * 128
                    return i * D + j * 128

                for j in range(xbar):
                    assert layout == "j"
                    nc.sync.dma_start_transpose(
                        xt_sb[:, j * JW:(j + 1) * JW].rearrange(
                            "p (i d) -> p i d", d=128),
                        x_g[:, j * GN + c2 * CHUNK:j * GN + (c2 + 1) * CHUNK],
                    )
                xt_ps = None
                if xbar < DJ:
                    xt_ps = xtps.tile([128, NI * D], F16, tag="xt",
                                      name="xt_ps")
                    for j in range(xbar, DJ):
                        for i in range(NI):
                            o = xt_off(j, i)
                            nc.tensor.transpose(
                                xt_ps[:, o:o + 128],
                                xblk(j, i),
                                id16[:],
                            )
                # softmax max + unnormalized exp
                nbias = stpool.tile([128, NI], F32, tag="nbias", name="nbias")
                nc.vector.tensor_reduce(
                    nbias[:],
                    score_ps[:].rearrange("p (i k) -> p i k", k=K),
                    axis=mybir.AxisListType.X,
                    op=ALU.max,
                    negate=True,
                )
                prob = ppool.tile([128, NI * K], F16, tag="prob", name="prob")
                for i in range(NI):
                    nc.scalar.activation(
                        prob[:, i * K:(i + 1) * K],
                        score_ps[:, i * K:(i + 1) * K],
                        AF.Exp,
                        bias=nbias[:, i:i + 1],
                    )
                return {"prob": prob, "xt_ps": xt_ps, "xt_sb": xt_sb}

            def stage_p2(b, c, st):
                """colsum, rcp, probn renorm, x^T evacuation.  Issued one
                slot behind stage_p1 so DVE's colsum (waiting on ACT exps)
                sits behind the NEXT chunk's max in the DVE queue."""
                if ablate == "compute":
                    return None
                prob, xt_ps = st["prob"], st["xt_ps"]
                xt_sb = st["xt_sb"]
                colsum = stpool.tile([128, NI], F32, tag="colsum",
                                     name="colsum")
                nc.vector.tensor_reduce(
                    colsum[:],
                    prob[:].rearrange("p (i k) -> p i k", k=K),
                    axis=mybir.AxisListType.X,
                    op=ALU.add,
                )
                rcp = stpool.tile([128, NI], F32, tag="rcp", name="rcp")
                nc.vector.reciprocal(rcp[:], colsum[:])
                probn = ppool.tile([128, NI * K], F16, tag="probn",
                                   name="probn")
                for i in range(NI):
                    nc.vector.tensor_scalar_mul(
                        probn[:, i * K:(i + 1) * K],
                        prob[:, i * K:(i + 1) * K],
                        rcp[:, i:i + 1])
                # evacuation of PE-transposed regions, split DVE/ACT.
                # (region granularity: j-blocks for layout=j, i-blocks for i)
                JW = NI * 128
                npe = DJ - xbar
                if npe > 0:
                    ndve = min(npe, evac_dve + (1 if c % 2 else 0))
                    lo = xbar if layout == "j" else 0
                    r = lo
                    while r < lo + ndve:
                        w = min(2, lo + ndve - r)
                        nc.vector.tensor_copy(
                            xt_sb[:, r * JW:(r + w) * JW],
                            xt_ps[:, r * JW:(r + w) * JW])
                        r += w
                    if lo + ndve < DJ:
                        nc.scalar.copy(
                            xt_sb[:, (lo + ndve) * JW:DJ * JW],
                            xt_ps[:, (lo + ndve) * JW:DJ * JW])
                return {"probn": probn, "xt_sb": xt_sb}

            def stage_q(b, c, st):
                if ablate == "compute":
                    return
                # NOTE: all five accumulation regions (4 desc j-slices + den)
                # live in ONE PSUM bank.  Interleaved start=True matmuls in a
                # shared bank wipe each other's first contribution (observed
                # on hw), so the bank is zeroed once by DVE memset and every
                # matmul accumulates with start=False.
                if c == 0:
                    shape = [128, D] if kd else [128, (DJ + 1) * K]
                    desc_t = accps.tile(shape, F32, tag="desc", name="desc_ps")
                    desc_state[b] = desc_t
                    nc.vector.memset(desc_t[:], 0.0)
                desc_t = desc_state[b]
                probn, xt_sb = st["probn"], st["xt_sb"]
                if kd:
                    # v1 orientation: out descT [k, d]; stationary probn is
                    # only 64 columns and streams 512 -> Ldweights fully
                    # hidden on hw; den [1, NI*K] on spare partition 64,
                    # one matmul per chunk (i-blocks summed at finalize).
                    last_c = (c == NCH - 1)
                    JW = NI * 128
                    for i in range(NI):
                        if layout == "j":
                            rhs = xt_sb[:].rearrange(
                                "p (j i d) -> p i j d", d=128, i=NI)[:, i]
                        else:
                            rhs = xt_sb[:, i * D:(i + 1) * D]
                        nc.tensor.matmul(
                            desc_t[0:K, 0:D],
                            probn[:, i * K:(i + 1) * K],
                            rhs,
                            start=False,
                            stop=(last_c and i == NI - 1),
                            skip_group_check=True,
                        )
                    nc.tensor.matmul(
                        desc_t[64:65, 0:NI * K],
                        ones16[:],
                        probn[:],
                        start=False,
                        stop=last_c,
                        skip_group_check=True,
                    )
                    return
                desc_ps = desc_t[:, 0:DJ * K]
                den_ps = desc_t[0:1, DJ * K:(DJ + 1) * K]
                JW = NI * 128
                for i in range(NI):
                    last = (c == NCH - 1 and i == NI - 1)
                    for j in range(DJ):
                        o = (j * JW + i * 128) if layout == "j" \
                            else (i * D + j * 128)
                        nc.tensor.matmul(
                            desc_ps[:, j * K:(j + 1) * K],
                            xt_sb[:, o:o + 128],
                            probn[:, i * K:(i + 1) * K],
                            start=False,
                            stop=last,
                            skip_group_check=True,
                        )
                    nc.tensor.matmul(
                        den_ps,
                        ones16[:],
                        probn[:, i * K:(i + 1) * K],
                        start=False,
                        stop=last,
                        skip_group_check=True,
                    )

            def finalize(b):
                if ablate == "compute":
                    return
                desc_t = desc_state[b]
                if kd:
                    # descT [k, d] -> evac, PE-transpose to [d, k], relocate
                    # den row from partition 64 to 0 via tiny SBUF-SBUF DMA.
                    descT_sb = fpool.tile([128, D], F32, tag="descT_sb")
                    nc.vector.tensor_copy(descT_sb[0:K, :], desc_t[0:K, 0:D])
                    nc.vector.tensor_reduce(
                        descT_sb[64:65, 0:K],
                        desc_t[64:65, 0:NI * K].rearrange(
                            "p (i k) -> p k i", k=K),
                        axis=mybir.AxisListType.X, op=ALU.add)
                    den0 = fpool.tile([1, K], F32, tag="den0")
                    nc.sync.dma_start(den0[:], descT_sb[64:65, 0:K])
                    dk_t = finps.tile([128, 2 * K], F32, tag="fin")
                    dk_sb = fpool.tile([128, DJ * K], F32, tag="desc_sb")
                    for j in range(DJ):
                        half = (j % 2) * K
                        nc.tensor.transpose(
                            dk_t[:, half:half + K],
                            descT_sb[0:K, j * 128:(j + 1) * 128],
                            id32[0:K, 0:K],
                        )
                        nc.vector.tensor_copy(dk_sb[:, j * K:(j + 1) * K],
                                              dk_t[:, half:half + K])
                    desc_sb = dk_sb
                    den_ps = den0[:]
                else:
                    desc_ps = desc_t[:, 0:DJ * K]
                    den_ps = desc_t[0:1, DJ * K:(DJ + 1) * K]
                    desc_sb = fpool.tile([128, DJ * K], F32, tag="desc_sb")
                    nc.vector.tensor_copy(desc_sb[:], desc_ps[:])
                if debug_acc:
                    dbg_sb = fpool.tile([128, (DJ + 1) * K], F32, tag="dbg")
                    nc.vector.tensor_copy(dbg_sb[:], desc_t[:])
                    nc.sync.dma_start(dbg[b], dbg_sb[:])
                den = fpool.tile([1, K], F32, tag="den_sb")
                nc.vector.tensor_scalar_max(den[:], den_ps, EPS)
                rden = fpool.tile([1, K], F32, tag="rden")
                nc.vector.reciprocal(rden[:], den[:])
                bc1_t = finps.tile([128, 2 * K], F32, tag="fin")
                bc1_ps = bc1_t[:, 0:K]
                nc.tensor.matmul(bc1_ps, ones_r1[:], rden[:],
                                 start=True, stop=True)
                # desc2 = desc/den - centers
                desc2 = fpool.tile([128, DJ * K], F32, tag="desc2")
                for j in range(DJ):
                    nc.vector.tensor_tensor(
                        desc2[:, j * K:(j + 1) * K],
                        desc_sb[:, j * K:(j + 1) * K],
                        bc1_ps, op=ALU.mult)
                nc.vector.tensor_tensor(desc2[:], desc2[:], ct[:],
                                        op=ALU.subtract)
                # intra-norm over d: ssq row via ones^T @ desc2^2
                sq = fpool.tile([128, DJ * K], F32, tag="sq")
                nc.scalar.activation(sq[:], desc2[:], AF.Square)
                ssq_t = finps.tile([128, 2 * K], F32, tag="fin")
                ssq_ps = ssq_t[0:1, 0:K]
                for j in range(DJ):
                    nc.tensor.matmul(ssq_ps, ones_c[:],
                                     sq[:, j * K:(j + 1) * K],
                                     start=(j == 0), stop=(j == DJ - 1))
                ssq = fpool.tile([1, K], F32, tag="ssq")
                nc.vector.tensor_scalar_max(ssq[:], ssq_ps, 1e-24)
                rn = fpool.tile([1, K], F32, tag="rn")
                nc.scalar.activation(rn[:], ssq[:], AF.Ln)
                nc.scalar.activation(rn[:], rn[:], AF.Exp, scale=-0.5)
                scl = fpool.tile([1, K], F32, tag="scl")
                nc.vector.tensor_tensor(scl[:], rn[:], cw_sb[:], op=ALU.mult)
                # global norm: gss = sum_k scl_k^2 * ssq_k
                s2 = fpool.tile([1, K], F32, tag="s2")
                nc.vector.tensor_tensor(s2[:], scl[:], scl[:], op=ALU.mult)
                nc.vector.tensor_tensor(s2[:], s2[:], ssq[:], op=ALU.mult)
                gss = fpool.tile([1, 1], F32, tag="gss")
                nc.vector.tensor_reduce(gss[:], s2[:],
                                        axis=mybir.AxisListType.X, op=ALU.add)
                nc.vector.tensor_scalar_max(gss[:], gss[:], 1e-24)
                grc = fpool.tile([1, 1], F32, tag="grc")
                nc.scalar.activation(grc[:], gss[:], AF.Ln)
                nc.scalar.activation(grc[:], grc[:], AF.Exp, scale=-0.5)
                total = fpool.tile([1, K], F32, tag="total")
                nc.vector.tensor_scalar_mul(total[:], scl[:], grc[0:1, 0:1])
                bc2_t = finps.tile([128, 2 * K], F32, tag="fin")
                bc2_ps = bc2_t[:, 0:K]
                nc.tensor.matmul(bc2_ps, ones_r1[:], total[:],
                                 start=True, stop=True)
                out_sb = fpool.tile([128, DJ * K], F32, tag="out_sb")
                for j in range(DJ):
                    nc.vector.tensor_tensor(
                        out_sb[:, j * K:(j + 1) * K],
                        desc2[:, j * K:(j + 1) * K],
                        bc2_ps, op=ALU.mult)
                nc.sync.dma_start(
                    out[b].rearrange("(j p k) -> p j k", p=128, k=K),
                    out_sb[:].rearrange("p (j k) -> p j k", k=K),
                )

            # driver: P(c) then Q(c-1); finalize(b) two slots after Q(b,last)
            import contextlib
            loop_cm = tc.For_i(0, reps, 1) if reps > 1 else contextlib.nullcontext()
            with loop_cm:
                seq = [(b, c) for b in range(B_LOC) for c in range(NCH)]
                st1, st2 = {}, {}
                sg = stagger
                for idx in range(len(seq) + 2 * sg + 1):
                    if idx < len(seq):
                        b, c = seq[idx]
                        st1[(b, c)] = stage_p1(b, c)
                    if sg <= idx < len(seq) + sg:
                        b, c = seq[idx - sg]
                        st2[(b, c)] = stage_p2(b, c, st1.pop((b, c)))
                    if 2 * sg <= idx < len(seq) + 2 * sg:
                        b, c = seq[idx - 2 * sg]
                        stage_q(b, c, st2.pop((b, c)))
                    if 2 * sg + 1 <= idx:
                        b, c = seq[idx - 2 * sg - 1]
                        if c == NCH - 1:
                            finalize(b)

    nc.compile()
    return nc


_NC_CACHE = [None]


def _make_in_maps(x, centers, alpha, cluster_weights):
    x = np.ascontiguousarray(np.asarray(x, dtype=np.float32))
    cen = np.ascontiguousarray(np.asarray(centers, dtype=np.float32).reshape(D, K))
    al = np.asarray(alpha, dtype=np.float32).reshape(1, 1)
    cw = np.ascontiguousarray(np.asarray(cluster_weights, dtype=np.float32).reshape(1, K))
    id16 = np.eye(128, dtype=np.float16)
    id32 = np.eye(128, dtype=np.float32)
    return [
        {
            "x_sh": x[core * B_LOC:(core + 1) * B_LOC],
            "centers": cen,
            "alpha": al,
            "cw": cw,
            "ident16": id16,
            "ident32": id32,
        }
        for core in range(NCORES)
    ]


def kernel(x, centers, alpha, cluster_weights):
    if _NC_CACHE[0] is None:
        _NC_CACHE[0] = _build()
    nc = _NC_CACHE[0]
    in_maps = _make_in_maps(x, centers, alpha, cluster_weights)
    res = run_bass_kernel_spmd(
        nc, in_maps, core_ids=list(range(NCORES)), trace=False
    )
    _run_results[0] = res
    out = np.concatenate([r["out"] for r in res.results], axis=0)
    return out.astype(np.float32)


def timed_run(x, centers, alpha, cluster_weights, iters=6, r_lo=1, r_hi=513):
    """Correctness output + For_i-amplified per-exec device time.

    Builds the kernel twice (reps=r_lo and reps=r_hi hardware loop around
    the main body), interleaves executions of both, and reports
    (wall_hi - wall_lo) / (r_hi - r_lo).  The axon per-dispatch overhead is
    identical for both builds and cancels in the differential.
    """
    import statistics
    import time
    import jax
    from jax.sharding import Mesh, PartitionSpec, NamedSharding
    from jax.experimental.shard_map import shard_map
    from concourse import bass2jax, mybir as mb

    inputs = dict(x=x, centers=centers, alpha=alpha,
                  cluster_weights=cluster_weights)
    bass2jax.install_neuronx_cc_hook()

    def make_runner(nc):
        in_maps = _make_in_maps(**inputs)
        partition_name = (nc.partition_id_tensor.name
                          if nc.partition_id_tensor else None)
        in_names, out_names, out_avals, zero_outs = [], [], [], []
        for alloc in nc.m.functions[0].allocations:
            if not isinstance(alloc, mb.MemoryLocationSet):
                continue
            name = alloc.memorylocations[0].name
            if alloc.kind == "ExternalInput":
                if name != partition_name:
                    in_names.append(name)
            elif alloc.kind == "ExternalOutput":
                out_names.append(name)
                shape = tuple(alloc.tensor_shape)
                dtype = mb.dt.np(alloc.dtype)
                out_avals.append(jax.core.ShapedArray(shape, dtype))
                zero_outs.append(np.zeros(shape, dtype))
        n_params = len(in_names)
        all_in_names = list(in_names) + list(out_names)
        if partition_name is not None:
            all_in_names.append(partition_name)

        def _body(*args):
            ins = list(args[:n_params])
            outs = list(args[n_params:])
            operands = ins + outs
            if partition_name is not None:
                operands.append(bass2jax.partition_id_tensor())
            return tuple(bass2jax._bass_exec_p.bind(
                *operands,
                out_avals=tuple(out_avals),
                in_names=tuple(all_in_names),
                out_names=tuple(out_names),
                lowering_input_output_aliases=(),
                sim_require_finite=True,
                sim_require_nnan=True,
                nc=nc,
            ))

        devices = jax.devices()[:NCORES]
        mesh = Mesh(np.asarray(devices), ("core",))
        spec = PartitionSpec("core")
        n_outs = len(out_avals)
        fn = jax.jit(shard_map(
            _body, mesh=mesh, in_specs=(spec,) * (n_params + n_outs),
            out_specs=(spec,) * n_outs, check_rep=False), keep_unused=True)

        per_core = [[np.asarray(m[name]) for name in in_names]
                    for m in in_maps]
        concat_in = [np.concatenate([per_core[c][i] for c in range(NCORES)],
                                    axis=0) for i in range(n_params)]
        concat_zeros = [np.zeros((NCORES * z.shape[0], *z.shape[1:]), z.dtype)
                        for z in zero_outs]
        sharding = NamedSharding(mesh, spec)
        staged = [jax.device_put(a, sharding) for a in concat_in]
        staged_zeros = [jax.device_put(a, sharding) for a in concat_zeros]
        jax.block_until_ready(staged)
        outs = fn(*staged, *staged_zeros)
        jax.block_until_ready(outs)
        oi = out_names.index("out")
        full_out = np.asarray(outs[oi]).reshape(B, D * K).astype(np.float32)

        def run():
            t0 = time.perf_counter()
            o = fn(*staged, *staged_zeros)
            jax.block_until_ready(o)
            return time.perf_counter() - t0
        return full_out, run

    out_lo, run_lo = make_runner(_build(reps=r_lo))
    _, run_hi = make_runner(_build(reps=r_hi))
    walls_lo, walls_hi = [], []
    for _ in range(iters):
        walls_lo.append(run_lo())
        walls_hi.append(run_hi())
    med = statistics.median
    per_med = (med(walls_hi) - med(walls_lo)) / (r_hi - r_lo)
    per_min = (min(walls_hi) - min(walls_lo)) / (r_hi - r_lo)
    return out_lo, {
        "per_exec_med": per_med, "per_exec_min": per_min,
        "walls_lo": walls_lo, "walls_hi": walls_hi,
        "r_lo": r_lo, "r_hi": r_hi,
    }
